# revision 29
# baseline (speedup 1.0000x reference)
"""Trainium2 Bass kernel for 2-layer single-head GAT (nn_GAT__80942953660642).

Strategy (8 NeuronCores, SPMD):
  - Nodes are assigned to cores by a balanced 4-coloring: each node gets a
    class w in 0..3 (class = core pair {2w, 2w+1}) chosen greedily so that
    every dst's in-neighbors spread evenly across classes. A class is one
    CONTIGUOUS 2*12544-row window of the AllGathered node table, so window
    rows fit int16 and the edge gather can use batched SWDGE dma_gather
    (994ns + 0.34ns/row per call) instead of per-slot indirect DMA
    (~1.04us per 128 rows), removing the GpSimd descriptor-generation wall.
  - Phase M: h_ext = X_shard @ [W0 | W0@al0 | W0@ar0] (fp16 in, fp32 PSUM);
    rows [h(140), el, er] packed into 256-fp16 rows (512B, dma_gather needs
    256B-multiple rows); er also kept in a resident SBUF strip.
  - AllGather -> full table; per dst-group (128 degree-sorted dsts/partition)
    edges are slotted per source window; chunks of groups are fetched with 4
    dma_gather calls (one per window, int16 window-relative indices).
    Softmax without max-subtraction; pad slots point at a sentinel row with
    el = -60000. Weighted accumulation via fused DVE multiply-add in fp16.
  - Layer 2 repeats with 128-fp16 rows [hp1(7), el1, er1]; same topology, so
    the same int16 index table drives both layers' gathers.
"""
import sys
sys.path.insert(0, "/opt/trn_rl_repo")
import numpy as np

N = 100000
NCORES = 8
SHARD = 12500
PSHARD = 12544          # 98 * 128
G = PSHARD // 128       # 98 groups
KDIM = 1536             # 1433 padded to 12*128
D0 = 140
D0E = 142               # h(140), el, er
D1 = 7
W0C = 256               # L0 table row (fp16): h(140), el(140), er(141), pad
W1C = 128               # L1 table row (fp16): hp1(7), el1(7), er1(8), pad
NW = 4                  # source windows (core pairs)
WROWS = 2 * PSHARD      # rows per window
CHCAP = 64              # max slots per gather chunk (both layers)
SUBK = 8                # max slot-columns (1024 idxs) per dma_gather call
SENT = np.float16(-60000.0)

_CACHE = {}
TRACE = False          # test harness sets this to capture an NTFF profile
LAST_EXEC_NS = None


def _color_nodes(src, dst):
    """Balanced greedy 4-coloring: every dst's in-edges spread over classes."""
    CAP = 2 * SHARD
    oe = np.argsort(src, kind="stable")
    ss = src[oe]
    dd = dst[oe]
    starts = np.searchsorted(ss, np.arange(N + 1))
    rng = np.random.default_rng(0)
    order = rng.permutation(N)
    cnt = np.zeros((N, NW), dtype=np.int32)
    color = np.full(N, -1, np.int8)
    used = np.zeros(NW, np.int64)
    NB = 200
    Bq = 1.6
    for b in range(NB):
        S = order[b * N // NB:(b + 1) * N // NB]
        lens = starts[S + 1] - starts[S]
        flat = np.concatenate(
            [np.arange(starts[s], starts[s + 1]) for s in S]) if lens.sum() else \
            np.zeros(0, np.int64)
        srcloc = np.repeat(np.arange(len(S)), lens)
        nbd = dd[flat]
        sc = np.zeros((len(S), NW))
        np.add.at(sc, srcloc, Bq ** cnt[nbd].astype(np.float64))
        sc += Bq ** cnt[S].astype(np.float64)
        sc[:, used >= CAP] = 1e18
        w = np.argmin(sc, axis=1)
        for wi in range(NW):
            sel = np.where(w == wi)[0]
            over = int(used[wi]) + len(sel) - CAP
            if over > 0:
                alt = np.argsort(sc[sel, wi])[::-1][:over]
                sc[sel[alt], wi] = 1e18
                w[sel[alt]] = np.argmin(sc[sel[alt]], axis=1)
        color[S] = w
        np.add.at(used, w, 1)
        cnt[S, w] += 1
        np.add.at(cnt, (nbd, w[srcloc]), 1)
    # exact-capacity repair
    for wi in range(NW):
        while used[wi] > CAP:
            cand = np.where(color == wi)[0]
            mv = cand[rng.integers(len(cand))]
            tgt = int(np.argmin(used))
            nb = dd[starts[mv]:starts[mv + 1]]
            cnt[nb, wi] -= 1; cnt[mv, wi] -= 1
            cnt[nb, tgt] += 1; cnt[mv, tgt] += 1
            color[mv] = tgt; used[wi] -= 1; used[tgt] += 1
    return color, cnt


def _host_prep(src, dst):
    src = np.asarray(src).astype(np.int64)
    dst = np.asarray(dst).astype(np.int64)
    deg = np.bincount(dst, minlength=N)
    color, cnt = _color_nodes(src, dst)

    # shards: class w -> cores 2w, 2w+1 (alternate after worst-case sort)
    node_lists = []
    for w in range(NW):
        nodes_w = np.where(color == w)[0]
        key = cnt[nodes_w].max(1).astype(np.int64) * 1000 + deg[nodes_w]
        nd = nodes_w[np.argsort(-key, kind="stable")]
        node_lists += [nd[0::2][:SHARD], nd[1::2][:SHARD]]
    # table position of each node (sorted space)
    pos = np.empty(N, dtype=np.int64)
    for c in range(NCORES):
        pos[node_lists[c]] = c * PSHARD + np.arange(SHARD)
    wnd = pos // WROWS                 # source window of each node
    wrel = pos - wnd * WROWS           # window-relative row (int16-safe)

    # shared per-(group, window) slot capacity
    Kgw = np.ones((G, NW), np.int64)
    for c in range(NCORES):
        c4 = np.concatenate([cnt[node_lists[c]],
                             np.zeros((PSHARD - SHARD, NW), np.int32)])
        Kgw = np.maximum(Kgw, c4.reshape(G, 128, NW).max(1))
    Kt = Kgw.sum(1)                    # slots per group

    # chunks of groups for batched gathers (shared by both layers)
    chunks, cur, s = [], [], 0
    for g in range(G):
        if cur and s + Kt[g] > CHCAP:
            chunks.append(cur); cur, s = [], 0
        cur.append(g); s += Kt[g]
    chunks.append(cur)

    percore = []
    for c in range(NCORES):
        nl = node_lists[c]
        rank = np.empty(N, dtype=np.int64)
        rank[nl] = np.arange(SHARD)
        m = np.isin(dst, nl)
        e_dst = dst[m]
        e_src = src[m]
        r = rank[e_dst]
        wsrc = wnd[e_src]
        ordr = np.lexsort((wsrc, r))
        r_s, w_s, src_s = r[ordr], wsrc[ordr], e_src[ordr]
        # kpos within (r, w)
        key = r_s * NW + w_s
        kpos = np.arange(len(key)) - np.searchsorted(key, key)
        # slot tables [PSHARD, NW, Kmax] window-relative, sentinel fill
        Kmax = int(Kgw.max())
        slots = np.full((PSHARD, NW, Kmax), PSHARD - 1, dtype=np.int64)
        assert (kpos < Kgw[np.minimum(r_s // 128, G - 1), w_s]).all()
        slots[r_s, w_s, kpos] = wrel[src_s]
        percore.append(dict(nl=nl, slots=slots))

    # pack per (chunk, w) wrapped int16 idx arrays -> big offs [128, TOTC]
    meta = []          # (w, chunk_index, colstart, Kchw)
    colstart = 0
    for (ci, ch) in enumerate(chunks):
        for w in range(NW):
            Kchw = int(sum(Kgw[g, w] for g in ch))
            meta.append((w, ci, colstart, Kchw))
            colstart += Kchw * 8
    totc = colstart

    for c in range(NCORES):
        slots = percore[c]["slots"]
        offs = np.zeros((16, totc), np.int16)
        for (w, ci, cst, Kchw) in meta:
            idxl = []
            for g in chunks[ci]:
                sg = slots[g * 128:(g + 1) * 128, w, :Kgw[g, w]]  # [128,K]
                idxl.append(sg.T)            # token j = c*128+p: k-major
            flat = np.concatenate(idxl, axis=0).reshape(-1)  # [Kchw*128]
            # wrapped: idx j at [j%16, j//16]
            wr = flat.reshape(-1, 16).T
            offs[:, cst:cst + Kchw * 8] = wr.astype(np.int16)
        # replicate the 16-partition pattern across all 128 partitions
        percore[c]["offs"] = np.tile(offs, (8, 1))
        del percore[c]["slots"]

    shared = dict(Kgw=Kgw, Kt=Kt, chunks=chunks, meta=meta, totc=totc)
    return percore, shared


def _prep_weights(inputs, percore):
    X = np.asarray(inputs["X"], np.float32)
    W0 = np.asarray(inputs["W0"], np.float64)
    al0 = np.asarray(inputs["al0"], np.float64)
    ar0 = np.asarray(inputs["ar0"], np.float64)
    b0 = np.asarray(inputs["b0"], np.float32)
    W1 = np.asarray(inputs["W1"], np.float64)
    al1 = np.asarray(inputs["al1"], np.float64)
    ar1 = np.asarray(inputs["ar1"], np.float64)
    b1 = np.asarray(inputs["b1"], np.float32)

    W0e = np.concatenate([W0, (W0 @ al0)[:, None], (W0 @ ar0)[:, None]], axis=1)
    W0p = np.zeros((KDIM, D0E), np.float16)
    W0p[:1433] = W0e.astype(np.float16)
    W0r = np.ascontiguousarray(
        W0p.reshape(12, 128, D0E).transpose(1, 0, 2).reshape(128, 12 * D0E))
    W1e = np.concatenate([W1, (W1 @ al1)[:, None], (W1 @ ar1)[:, None]],
                         axis=1).astype(np.float16)
    W1a = np.zeros((128, D1 + 2), np.float16)
    W1a[:128] = W1e[:128]
    W1b = np.zeros((128, D1 + 2), np.float16)
    W1b[:12] = W1e[128:140]
    bc = lambda v, w, dt: np.broadcast_to(np.asarray(v, dt)[None, :],
                                          (128, w)).copy()
    ident = np.eye(128, dtype=np.float16)
    sent_mask = np.zeros((128, 1), np.float16)
    sent_mask[SHARD - (G - 1) * 128:, 0] = SENT   # partitions 84.. are pads
    com = dict(W0r=W0r, W1a=W1a, W1b=W1b, sent_mask=sent_mask,
               b0b=bc(b0, D0, np.float16), b1b=bc(b1, D1, np.float32),
               ident=ident)

    xts = []
    for c in range(NCORES):
        nl = percore[c]["nl"]
        Xp = np.zeros((PSHARD, KDIM), np.float16)
        Xp[:SHARD, :1433] = X[nl, :].astype(np.float16)
        xt = Xp.reshape(G, 128, 12, 128).transpose(0, 3, 2, 1).reshape(G, 128, 12 * 128)
        xt = np.concatenate([xt, np.zeros((100 - G, 128, 12 * 128), np.float16)])
        xt = np.ascontiguousarray(
            xt.reshape(50, 2, 128, 1536).transpose(0, 2, 1, 3).reshape(50, 128, 2 * 1536))
        xts.append(xt)
    return com, xts


BISECT = False


def _build(shared):
    import concourse.bass as bass
    import concourse.tile as tile
    from concourse import bacc, mybir, library_config
    dt = mybir.dt
    op = mybir.AluOpType
    act = mybir.ActivationFunctionType

    Kgw = shared["Kgw"]
    totc = shared["totc"]
    nc = bacc.Bacc("TRN2", target_bir_lowering=False, debug=False,
                   num_devices=NCORES)
    t_x = nc.dram_tensor("x_up", [50, 128, 2 * 12 * 128], dt.float16, kind="ExternalInput")
    t_w0 = nc.dram_tensor("w0r", [128, 12 * D0E], dt.float16, kind="ExternalInput")
    t_w1a = nc.dram_tensor("w1a", [128, D1 + 2], dt.float16, kind="ExternalInput")
    t_w1b = nc.dram_tensor("w1b", [128, D1 + 2], dt.float16, kind="ExternalInput")
    t_b0 = nc.dram_tensor("b0b", [128, D0], dt.float16, kind="ExternalInput")
    t_b1 = nc.dram_tensor("b1b", [128, D1], dt.float32, kind="ExternalInput")
    t_id = nc.dram_tensor("ident", [128, 128], dt.float16, kind="ExternalInput")
    t_of = nc.dram_tensor("offs", [128, totc], dt.int16, kind="ExternalInput")
    t_sm = nc.dram_tensor("sent_mask", [128, 1], dt.float16, kind="ExternalInput")
    t_out = nc.dram_tensor("out_buf", [PSHARD, D1], dt.float32, kind="ExternalOutput")

    with tile.TileContext(nc) as tc:
        with tc.tile_pool(name="const", bufs=1) as cpool, \
             tc.tile_pool(name="xload", bufs=2) as xpool, \
             tc.tile_pool(name="hex", bufs=3) as hexpool, \
             tc.tile_pool(name="gath", bufs=2) as gpool, \
             tc.tile_pool(name="work", bufs=3) as wpool, \
             tc.tile_pool(name="small", bufs=4) as spool, \
             tc.tile_pool(name="psum", bufs=2, space="PSUM") as ppool, \
             tc.tile_pool(name="dram", bufs=1, space="DRAM") as dpool:

            w0_sb = cpool.tile([128, 12 * D0E], dt.float16)
            nc.sync.dma_start(w0_sb[:], t_w0[:])
            w1a_sb = cpool.tile([128, D1 + 2], dt.float16)
            nc.sync.dma_start(w1a_sb[:], t_w1a[:])
            w1b_sb = cpool.tile([128, D1 + 2], dt.float16)
            nc.sync.dma_start(w1b_sb[:], t_w1b[:])
            b0_sb = cpool.tile([128, D0], dt.float16)
            nc.sync.dma_start(b0_sb[:], t_b0[:])
            b1_sb = cpool.tile([128, D1], dt.float32)
            nc.sync.dma_start(b1_sb[:], t_b1[:])
            id_sb = cpool.tile([128, 128], dt.float16)
            nc.sync.dma_start(id_sb[:], t_id[:])
            of_sb = cpool.tile([128, totc], dt.int16)
            nc.sync.dma_start(of_sb[:], t_of[:])
            sm_sb = cpool.tile([128, 1], dt.float16)
            nc.sync.dma_start(sm_sb[:], t_sm[:])
            er0_all = cpool.tile([128, G], dt.float32)
            er1_all = cpool.tile([128, G], dt.float32)

            shard0 = dpool.tile([PSHARD, W0C], dt.float16)
            table0 = dpool.tile([NCORES * PSHARD, W0C], dt.float16, addr_space="Shared")
            shard1 = dpool.tile([PSHARD, W1C], dt.float16)
            table1 = dpool.tile([NCORES * PSHARD, W1C], dt.float16, addr_space="Shared")

            # manual DMA-completion sem for dma_gather (not an InstDMA subclass,
            # so Tile only syncs on the engine / desc-gen completion)
            nc.gpsimd.load_library(library_config.mlp)

            # ---- Phase M ----
            XB = 2
            xts = {}
            for n in range(G):
                b, t = n // XB, n % XB
                if t == 0:
                    xtile = xpool.tile([128, XB * 12 * 128], dt.float16, tag="xt")
                    xts[b] = xtile
                    nc.sync.dma_start(xtile[:], t_x[:][b])
                xt = xts[b][:, t * 1536:(t + 1) * 1536]
                ph = ppool.tile([128, D0E], dt.float32, space="PSUM")
                for k in range(12):
                    nc.tensor.matmul(ph[:], xt[:, k * 128:(k + 1) * 128],
                                     w0_sb[:, k * D0E:(k + 1) * D0E],
                                     start=(k == 0), stop=(k == 11))
                hx = hexpool.tile([128, W0C], dt.float16, tag="hex0")
                nc.vector.tensor_copy(hx[:, 0:D0E], ph[:])
                nc.vector.memset(hx[:, D0E:W0C], 0.0)
                nc.vector.tensor_copy(er0_all[:, n:n + 1], ph[:, 141:142])
                if n == G - 1:
                    nc.vector.tensor_tensor(hx[:, 140:141], hx[:, 140:141],
                                            sm_sb[:], op=op.add)
                nc.sync.dma_start(shard0[:].rearrange("(g p) w -> g p w", p=128)[n],
                                  hx[:])

            nc.gpsimd.collective_compute(
                "AllGather", op.bypass, replica_groups=[list(range(NCORES))],
                ins=[shard0[:]], outs=[table0[:]])

            # ---- Phase E0 ----
            t0w = table0[:].rearrange("(v r) e -> v r e", v=NW)
            for (ci, ch) in enumerate(shared["chunks"]):
                KC = int(sum(shared["Kt"][g] for g in ch))
                gt = gpool.tile([128, KC * W0C], dt.float16, tag="g0")
                gv = gt[:].rearrange("p (c e) -> p c e", e=W0C)
                # column layout: [w][g][k]; per-(g) slot order: w-major
                wbase = {}
                cofs = 0
                for w in range(NW):
                    wbase[w] = cofs
                    cofs += int(sum(Kgw[g, w] for g in ch))
                mrun = [m for m in shared["meta"] if m[1] == ci]
                for (w, _ci, colstart, Kchw) in mrun:
                    for q0 in range(0, Kchw, SUBK):
                        qk = min(SUBK, Kchw - q0)
                        nc.gpsimd.dma_gather(
                            out_ap=gv[:, wbase[w] + q0:wbase[w] + q0 + qk],
                            in_ap=t0w[w],
                            idxs_ap=of_sb[0:16, colstart + q0 * 8:
                                          colstart + (q0 + qk) * 8],
                            num_idxs=qk * 128, num_idxs_reg=qk * 128,
                            elem_size=W0C)
                # ring-flush barrier: this InstDMACopy's descriptors sit behind
                # the gathers' in the SWDGE rings; Tile syncs readers on it
                flg = spool.tile([128, 16], dt.float16, tag="flg0")
                nc.gpsimd.dma_start(flg[:], gt[:, 0:16])
                fjk = spool.tile([128, 1], dt.float32, tag="fjk0")
                nc.vector.tensor_copy(fjk[:], flg[:, 0:1])
                # per-group compute
                goff = {w: wbase[w] for w in range(NW)}
                for g in ch:
                    K = int(shared["Kt"][g])
                    cols = []
                    for w in range(NW):
                        cols += list(range(goff[w], goff[w] + int(Kgw[g, w])))
                        goff[w] += int(Kgw[g, w])
                    ep = spool.tile([128, K], dt.float32, tag="ep0")
                    s = 0
                    for w in range(NW):
                        kw = int(Kgw[g, w])
                        a = cols[s]
                        nc.vector.tensor_scalar(ep[:, s:s + kw],
                                                gv[:, a:a + kw, 140],
                                                er0_all[:, g:g + 1], None, op.add)
                        s += kw
                    ee = spool.tile([128, K], dt.float32, tag="ee0")
                    nc.vector.scalar_tensor_tensor(
                        out=ee[:], in0=ep[:], scalar=0.2, in1=ep[:],
                        op0=op.mult, op1=op.max)
                    ex = spool.tile([128, K], dt.float32, tag="ex0")
                    dn = spool.tile([128, 1], dt.float32, tag="dn0")
                    nc.scalar.activation(ex[:], ee[:], act.Exp, accum_out=dn[:])
                    nc.vector.tensor_scalar_max(dn[:], dn[:], 1e-30)
                    rv = spool.tile([128, 1], dt.float32, tag="rv0")
                    nc.vector.reciprocal(rv[:], dn[:])
                    acc = wpool.tile([128, D0], dt.float16, tag="acc0")
                    nc.vector.tensor_scalar(acc[:], gv[:, cols[0], 0:D0],
                                            ex[:, 0:1], None, op.mult)
                    for k in range(1, K):
                        nc.vector.scalar_tensor_tensor(
                            out=acc[:], in0=gv[:, cols[k], 0:D0],
                            scalar=ex[:, k:k + 1],
                            in1=acc[:], op0=op.mult, op1=op.add)
                    h1 = wpool.tile([128, D0], dt.float16, tag="h1")
                    nc.vector.scalar_tensor_tensor(
                        out=h1[:], in0=acc[:], scalar=rv[:], in1=b0_sb[:],
                        op0=op.mult, op1=op.add)
                    nc.scalar.activation(h1[:], h1[:], act.Relu)
                    if BISECT:
                        otb = spool.tile([128, D1], dt.float32, tag="otb")
                        nc.vector.tensor_copy(otb[:], h1[:, 0:D1])
                        nc.sync.dma_start(
                            t_out[:].rearrange("(g p) w -> g p w", p=128)[g], otb[:])
                        continue
                    pt1 = ppool.tile([128, 128], dt.float16, space="PSUM", tag="pt1")
                    nc.tensor.transpose(pt1[:], h1[:, 0:128], id_sb[:])
                    pt2 = ppool.tile([128, 128], dt.float16, space="PSUM", tag="pt2")
                    nc.tensor.transpose(pt2[0:12, :], h1[:, 128:140], id_sb[:])
                    t1s = wpool.tile([128, 128], dt.float16, tag="t1s")
                    nc.vector.tensor_copy(t1s[:], pt1[:])
                    t2s = wpool.tile([128, 128], dt.float16, tag="t2s")
                    nc.vector.tensor_copy(t2s[0:12, :], pt2[0:12, :])
                    php = ppool.tile([128, D1 + 2], dt.float32, space="PSUM", tag="php")
                    nc.tensor.matmul(php[:], t1s[:], w1a_sb[:], start=True, stop=False)
                    nc.tensor.matmul(php[:], t2s[0:12, :], w1b_sb[0:12, :],
                                     start=False, stop=True)
                    hx1 = hexpool.tile([128, W1C], dt.float16, tag="hex1")
                    nc.vector.tensor_copy(hx1[:, 0:D1 + 2], php[:])
                    nc.vector.memset(hx1[:, D1 + 2:W1C], 0.0)
                    if g == G - 1:
                        nc.vector.tensor_tensor(hx1[:, 7:8], hx1[:, 7:8],
                                                sm_sb[:], op=op.add)
                    nc.vector.tensor_copy(er1_all[:, g:g + 1], php[:, 8:9])
                    nc.sync.dma_start(
                        shard1[:].rearrange("(g p) w -> g p w", p=128)[g], hx1[:])

            if not BISECT:
                nc.gpsimd.collective_compute(
                    "AllGather", op.bypass, replica_groups=[list(range(NCORES))],
                    ins=[shard1[:]], outs=[table1[:]])

            # ---- Phase E1 ----
            t1w = table1[:].rearrange("(v r) e -> v r e", v=NW)
            for (ci, ch) in enumerate(shared["chunks"] if not BISECT else []):
                KC = int(sum(shared["Kt"][g] for g in ch))
                gt = gpool.tile([128, KC * W1C], dt.float16, tag="g1")
                gv = gt[:].rearrange("p (c e) -> p c e", e=W1C)
                wbase = {}
                cofs = 0
                for w in range(NW):
                    wbase[w] = cofs
                    cofs += int(sum(Kgw[g, w] for g in ch))
                mrun = [m for m in shared["meta"] if m[1] == ci]
                for (w, _ci, colstart, Kchw) in mrun:
                    for q0 in range(0, Kchw, SUBK):
                        qk = min(SUBK, Kchw - q0)
                        nc.gpsimd.dma_gather(
                            out_ap=gv[:, wbase[w] + q0:wbase[w] + q0 + qk],
                            in_ap=t1w[w],
                            idxs_ap=of_sb[0:16, colstart + q0 * 8:
                                          colstart + (q0 + qk) * 8],
                            num_idxs=qk * 128, num_idxs_reg=qk * 128,
                            elem_size=W1C)
                flg = spool.tile([128, 16], dt.float16, tag="flg1")
                nc.gpsimd.dma_start(flg[:], gt[:, 0:16])
                fjk = spool.tile([128, 1], dt.float32, tag="fjk1")
                nc.vector.tensor_copy(fjk[:], flg[:, 0:1])
                goff = {w: wbase[w] for w in range(NW)}
                for g in ch:
                    K = int(shared["Kt"][g])
                    cols = []
                    for w in range(NW):
                        cols += list(range(goff[w], goff[w] + int(Kgw[g, w])))
                        goff[w] += int(Kgw[g, w])
                    ep = spool.tile([128, K], dt.float32, tag="ep1")
                    s = 0
                    for w in range(NW):
                        kw = int(Kgw[g, w])
                        a = cols[s]
                        nc.vector.tensor_scalar(ep[:, s:s + kw],
                                                gv[:, a:a + kw, 7],
                                                er1_all[:, g:g + 1], None, op.add)
                        s += kw
                    ee = spool.tile([128, K], dt.float32, tag="ee1")
                    nc.vector.scalar_tensor_tensor(
                        out=ee[:], in0=ep[:], scalar=0.2, in1=ep[:],
                        op0=op.mult, op1=op.max)
                    ex = spool.tile([128, K], dt.float32, tag="ex1")
                    dn = spool.tile([128, 1], dt.float32, tag="dn1")
                    nc.scalar.activation(ex[:], ee[:], act.Exp, accum_out=dn[:])
                    nc.vector.tensor_scalar_max(dn[:], dn[:], 1e-30)
                    rv = spool.tile([128, 1], dt.float32, tag="rv1")
                    nc.vector.reciprocal(rv[:], dn[:])
                    acc = spool.tile([128, D1], dt.float16, tag="acc1")
                    nc.vector.tensor_scalar(acc[:], gv[:, cols[0], 0:D1],
                                            ex[:, 0:1], None, op.mult)
                    for k in range(1, K):
                        nc.vector.scalar_tensor_tensor(
                            out=acc[:], in0=gv[:, cols[k], 0:D1],
                            scalar=ex[:, k:k + 1],
                            in1=acc[:], op0=op.mult, op1=op.add)
                    ot = spool.tile([128, D1], dt.float32, tag="ot")
                    nc.vector.scalar_tensor_tensor(
                        out=ot[:], in0=acc[:], scalar=rv[:], in1=b1_sb[:],
                        op0=op.mult, op1=op.add)
                    nc.scalar.activation(ot[:], ot[:], act.Relu)
                    nc.sync.dma_start(
                        t_out[:].rearrange("(g p) w -> g p w", p=128)[g], ot[:])
    nc.compile()
    return nc


def kernel(**inputs):
    percore, shared = _host_prep(inputs["src"], inputs["dst"])
    com, xts = _prep_weights(inputs, percore)

    key = (tuple(shared["Kt"]), tuple(map(tuple, shared["Kgw"])))
    if key not in _CACHE:
        _CACHE[key] = _build(shared)
    nc = _CACHE[key]

    in_maps = []
    for c in range(NCORES):
        pc = percore[c]
        m = dict(x_up=xts[c], w0r=com["W0r"], w1a=com["W1a"], w1b=com["W1b"],
                 b0b=com["b0b"], b1b=com["b1b"],
                 ident=com["ident"], offs=pc["offs"],
                 sent_mask=com["sent_mask"])
        in_maps.append(m)

    from concourse.bass_utils import run_bass_kernel_spmd
    global LAST_EXEC_NS
    res = run_bass_kernel_spmd(nc, in_maps, core_ids=list(range(NCORES)),
                               trace=TRACE)
    LAST_EXEC_NS = res.exec_time_ns
    out = np.zeros((N, D1), dtype=np.float32)
    for c in range(NCORES):
        ob = res.results[c]["out_buf"]
        out[percore[c]["nl"]] = ob[:SHARD]
    return out


# revision 30
# speedup vs baseline: 1.2721x; 1.2721x over previous
"""Trainium2 Bass kernel for 2-layer single-head GAT (nn_GAT__80942953660642).

Strategy (8 NeuronCores, SPMD):
  - Nodes sharded contiguously: core c owns nodes [c*12500, (c+1)*12500).
  - Phase M: h_ext = X_shard @ [W0 | W0@al0 | W0@ar0] on PE (fp16 inputs,
    fp32 PSUM) -> rows [h(140), el, er, pad] packed into a 144-fp16 shard
    table.
  - AllGather the fp16 shard tables -> full node table per core.
  - Phase E0 (edge phase): per core, its dst nodes are degree-sorted into 98
    groups of 128 (one dst per partition). Each dst's incoming edges occupy
    padded slot columns; ALL K slots of a group are fetched with ONE batched
    indirect DMA (offset AP [128, K] -> out [128, K*144]), amortizing the
    ~1us SWDGE fixed cost. Edge softmax without max-subtraction; padding
    slots point at sentinel rows with el = -60000 so exp() kills them.
    Weighted accumulation via fused DVE multiply-add (fp16 data, fp32
    per-partition scalars).
  - hp1/el1/er1 = h1 @ [W1 | W1@al1 | W1@ar1] via PE transpose + matmul,
    second 16-fp16 table, AllGather, Phase E1 repeats at width 7.
  - Host assembles the final [100000, 7] fp32 output (inverse degree-sort).
"""
import sys
sys.path.insert(0, "/opt/trn_rl_repo")
import numpy as np

N = 100000
NCORES = 8
SHARD = 12500
PSHARD = 12544          # 98 * 128
G = PSHARD // 128       # 98 groups
KDIM = 1536             # 1433 padded to 12*128
D0 = 140
D0E = 142               # h(140), el, er
D1 = 7
W0C = 144               # L0 table row: h(140), el(140), er(141), pad
W1C = 16                # L1 table row: hp1(7), el1(7), er1(8), pad
SENT = np.float16(-60000.0)

_CACHE = {}
TRACE = False          # test harness sets this to capture an NTFF profile
LAST_EXEC_NS = None


def _host_prep(src, dst):
    src = np.asarray(src).astype(np.int64)
    dst = np.asarray(dst).astype(np.int64)
    deg = np.bincount(dst, minlength=N)
    nodes = np.arange(N, dtype=np.int64)
    pad_id = (nodes // SHARD) * PSHARD + (nodes % SHARD)  # original-order padded id

    percore = []
    Kg = np.zeros(G, dtype=np.int64)
    for c in range(NCORES):
        lo = c * SHARD
        m = (dst >= lo) & (dst < lo + SHARD)
        e_dst = dst[m] - lo
        e_src = src[m]
        d = deg[lo:lo + SHARD]
        order = np.argsort(-d, kind="stable")
        rank = np.empty(SHARD, dtype=np.int64)
        rank[order] = np.arange(SHARD)
        dsort = np.concatenate([d[order], np.zeros(PSHARD - SHARD, np.int64)])
        for g in range(G):
            Kg[g] = max(Kg[g], max(1, dsort[g * 128:(g + 1) * 128].max()))
        percore.append(dict(order=order, rank=rank, e_dst=e_dst, e_src=e_src))

    pos1 = np.empty(N, dtype=np.int64)  # sorted-space padded id
    for c in range(NCORES):
        pos1[c * SHARD:(c + 1) * SHARD] = c * PSHARD + percore[c]["rank"]

    SK = int(Kg.sum())
    cums = np.concatenate([[0], np.cumsum(Kg)])
    for c in range(NCORES):
        pc = percore[c]
        sent_row = c * PSHARD + PSHARD - 1
        r = pc["rank"][pc["e_dst"]]
        # self-loop edges first within each dst -> they land in slot 0
        not_self = (pc["e_src"] != pc["e_dst"] + c * SHARD).astype(np.int64)
        ordr = np.lexsort((not_self, r))
        r_s = r[ordr]
        kpos = np.arange(len(r_s)) - np.searchsorted(r_s, r_s)
        Kcap = int(Kg.max())
        slots0 = np.full((PSHARD, Kcap), sent_row, dtype=np.int64)
        slots0[r_s, kpos] = pos1[pc["e_src"][ordr]]
        slots1 = np.full((PSHARD, Kcap), sent_row, dtype=np.int64)
        slots1[r_s, kpos] = pos1[pc["e_src"][ordr]]
        # pack per-group [128, Kg[g]] -> [128, SK] (row offsets)
        offs0 = np.zeros((128, SK), np.int32)
        offs1 = np.zeros((128, SK), np.int32)
        for g in range(G):
            offs0[:, cums[g]:cums[g + 1]] = slots0[g * 128:(g + 1) * 128, :Kg[g]]
            offs1[:, cums[g]:cums[g + 1]] = slots1[g * 128:(g + 1) * 128, :Kg[g]]
        pc["offs0"] = offs0
        pc["offs1"] = offs1
    return percore, Kg.astype(int), cums.astype(int)


def _prep_weights(inputs, percore):
    X = np.asarray(inputs["X"], np.float32)
    W0 = np.asarray(inputs["W0"], np.float64)
    al0 = np.asarray(inputs["al0"], np.float64)
    ar0 = np.asarray(inputs["ar0"], np.float64)
    b0 = np.asarray(inputs["b0"], np.float32)
    W1 = np.asarray(inputs["W1"], np.float64)
    al1 = np.asarray(inputs["al1"], np.float64)
    ar1 = np.asarray(inputs["ar1"], np.float64)
    b1 = np.asarray(inputs["b1"], np.float32)

    # W0ext = [W0 | W0@al0 | W0@ar0]: h, el, er from one matmul
    W0e = np.concatenate([W0, (W0 @ al0)[:, None], (W0 @ ar0)[:, None]], axis=1)
    W0p = np.zeros((KDIM, D0E), np.float16)
    W0p[:1433] = W0e.astype(np.float16)
    # W0r[kp, k*142+j] = W0p[k*128+kp, j]
    W0r = np.ascontiguousarray(
        W0p.reshape(12, 128, D0E).transpose(1, 0, 2).reshape(128, 12 * D0E))
    # W1ext = [W1 | W1@al1 | W1@ar1] -> [140, 9]
    W1e = np.concatenate([W1, (W1 @ al1)[:, None], (W1 @ ar1)[:, None]],
                         axis=1).astype(np.float16)
    W1a = np.zeros((128, D1 + 2), np.float16)
    W1a[:128] = W1e[:128]
    W1b = np.zeros((128, D1 + 2), np.float16)
    W1b[:12] = W1e[128:140]
    bc = lambda v, w, dt: np.broadcast_to(np.asarray(v, dt)[None, :],
                                          (128, w)).copy()
    ident = np.eye(128, dtype=np.float16)
    sent_mask = np.zeros((128, 1), np.float16)
    sent_mask[SHARD - (G - 1) * 128:, 0] = SENT   # partitions 84.. are pads
    com = dict(W0r=W0r, W1a=W1a, W1b=W1b, sent_mask=sent_mask,
               b0b=bc(b0, D0, np.float16), b1b=bc(b1, D1, np.float32),
               ident=ident)

    # X tiles per core (fp16): xt[n, kp, k*128+nf] = X[lo + n*128+nf, k*128+kp]
    xts = []
    for c in range(NCORES):
        lo = c * SHARD
        Xp = np.zeros((PSHARD, KDIM), np.float16)
        Xp[:SHARD, :1433] = X[lo + percore[c]['order'], :].astype(np.float16)
        xt = Xp.reshape(G, 128, 12, 128).transpose(0, 3, 2, 1).reshape(G, 128, 12 * 128)
        xt = np.concatenate([xt, np.zeros((100 - G, 128, 12 * 128), np.float16)])
        xt = np.ascontiguousarray(
            xt.reshape(25, 4, 128, 1536).transpose(0, 2, 1, 3).reshape(25, 128, 4 * 1536))
        xts.append(xt)
    return com, xts


def _build(Kg, cums):
    import concourse.bass as bass
    import concourse.tile as tile
    from concourse import bacc, mybir
    dt = mybir.dt
    op = mybir.AluOpType
    act = mybir.ActivationFunctionType

    SK = int(sum(Kg))
    nc = bacc.Bacc("TRN2", target_bir_lowering=False, debug=False,
                   num_devices=NCORES)
    t_x = nc.dram_tensor("x_up", [25, 128, 4 * 12 * 128], dt.float16, kind="ExternalInput")
    t_w0 = nc.dram_tensor("w0r", [128, 12 * D0E], dt.float16, kind="ExternalInput")
    t_w1a = nc.dram_tensor("w1a", [128, D1 + 2], dt.float16, kind="ExternalInput")
    t_w1b = nc.dram_tensor("w1b", [128, D1 + 2], dt.float16, kind="ExternalInput")
    t_b0 = nc.dram_tensor("b0b", [128, D0], dt.float16, kind="ExternalInput")
    t_b1 = nc.dram_tensor("b1b", [128, D1], dt.float32, kind="ExternalInput")
    t_id = nc.dram_tensor("ident", [128, 128], dt.float16, kind="ExternalInput")
    t_of0 = nc.dram_tensor("offs0", [128, SK], dt.int32, kind="ExternalInput")
    t_of1 = nc.dram_tensor("offs1", [128, SK], dt.int32, kind="ExternalInput")
    t_sm = nc.dram_tensor("sent_mask", [128, 1], dt.float16, kind="ExternalInput")
    t_out = nc.dram_tensor("out_buf", [PSHARD, D1], dt.float32, kind="ExternalOutput")

    with tile.TileContext(nc) as tc:
        with tc.tile_pool(name="const", bufs=1) as cpool, \
             tc.tile_pool(name="xload", bufs=2) as xpool, \
             tc.tile_pool(name="hex", bufs=3) as hexpool, \
             tc.tile_pool(name="gath", bufs=4) as gpool, \
             tc.tile_pool(name="work", bufs=3) as wpool, \
             tc.tile_pool(name="small", bufs=6) as spool, \
             tc.tile_pool(name="psum", bufs=2, space="PSUM") as ppool, \
             tc.tile_pool(name="dram", bufs=1, space="DRAM") as dpool:

            # constants
            w0_sb = cpool.tile([128, 12 * D0E], dt.float16)
            nc.sync.dma_start(w0_sb[:], t_w0[:])
            w1a_sb = cpool.tile([128, D1 + 2], dt.float16)
            nc.sync.dma_start(w1a_sb[:], t_w1a[:])
            w1b_sb = cpool.tile([128, D1 + 2], dt.float16)
            nc.sync.dma_start(w1b_sb[:], t_w1b[:])
            b0_sb = cpool.tile([128, D0], dt.float16)
            nc.sync.dma_start(b0_sb[:], t_b0[:])
            b1_sb = cpool.tile([128, D1], dt.float32)
            nc.sync.dma_start(b1_sb[:], t_b1[:])
            id_sb = cpool.tile([128, 128], dt.float16)
            nc.sync.dma_start(id_sb[:], t_id[:])
            of0_sb = cpool.tile([128, SK], dt.int32)
            nc.sync.dma_start(of0_sb[:], t_of0[:])
            of1_sb = cpool.tile([128, SK], dt.int32)
            nc.sync.dma_start(of1_sb[:], t_of1[:])
            sm_sb = cpool.tile([128, 1], dt.float16)
            nc.sync.dma_start(sm_sb[:], t_sm[:])
            er1_all = cpool.tile([128, G], dt.float32)

            shard0 = dpool.tile([PSHARD, W0C], dt.float16)
            table0 = dpool.tile([NCORES * PSHARD, W0C], dt.float16, addr_space="Shared")
            shard1 = dpool.tile([PSHARD, W1C], dt.float16)
            table1 = dpool.tile([NCORES * PSHARD, W1C], dt.float16, addr_space="Shared")

            # ---- Phase M: h_ext = X @ W0ext, pack [h, el, er] rows ----
            XB = 4                      # X tiles per DMA (amortize fixed cost)
            xts = {}
            for n in range(G):
                b, t = n // XB, n % XB
                if t == 0:
                    xtile = xpool.tile([128, 4 * 12 * 128], dt.float16, tag="xt")
                    xts[b] = xtile
                    nc.sync.dma_start(xtile[:], t_x[:][b])
                xt = xts[b][:, t * 1536:(t + 1) * 1536]
                ph = ppool.tile([128, D0E], dt.float32, space="PSUM")
                for k in range(12):
                    nc.tensor.matmul(ph[:], xt[:, k * 128:(k + 1) * 128],
                                     w0_sb[:, k * D0E:(k + 1) * D0E],
                                     start=(k == 0), stop=(k == 11))
                hx = hexpool.tile([128, W0C], dt.float16, tag="hex0")
                nc.vector.tensor_copy(hx[:, 0:D0E], ph[:])
                nc.vector.memset(hx[:, D0E:W0C], 0.0)
                if n == G - 1:
                    nc.vector.tensor_tensor(hx[:, 140:141], hx[:, 140:141],
                                            sm_sb[:], op=op.add)
                nc.sync.dma_start(shard0[:].rearrange("(g p) w -> g p w", p=128)[n],
                                  hx[:])

            nc.gpsimd.collective_compute(
                "AllGather", op.bypass, replica_groups=[list(range(NCORES))],
                ins=[shard0[:]], outs=[table0[:]])

            # ---- Phase E0 ----
            for g in range(G):
                K = int(Kg[g])
                gt = gpool.tile([128, K * W0C], dt.float16, tag="g0")
                gv = gt[:].rearrange("p (k w) -> p k w", w=W0C)
                # slot 0 is the self-loop: contiguous rows of our own shard
                nc.sync.dma_start(
                    gv[:, 0], shard0[:].rearrange("(g p) w -> g p w", p=128)[g])
                for k in range(1, K):
                    nc.gpsimd.indirect_dma_start(
                        out=gv[:, k], out_offset=None, in_=table0[:],
                        in_offset=bass.IndirectOffsetOnAxis(
                            ap=of0_sb[:, cums[g] + k:cums[g] + k + 1], axis=0))
                # slot 0 is the self-loop -> its row IS the dst row; er = col 141
                er0 = spool.tile([128, 1], dt.float32, tag="er0")
                nc.vector.tensor_copy(er0[:], gv[:, 0, 141:142])
                ep = spool.tile([128, K], dt.float32, tag="ep0")
                nc.vector.tensor_scalar(ep[:], gv[:, :, 140], er0[:],
                                        None, op.add)
                ee = spool.tile([128, K], dt.float32, tag="ee0")
                nc.vector.scalar_tensor_tensor(
                    out=ee[:], in0=ep[:], scalar=0.2, in1=ep[:],
                    op0=op.mult, op1=op.max)
                ex = spool.tile([128, K], dt.float32, tag="ex0")
                dn = spool.tile([128, 1], dt.float32, tag="dn0")
                nc.scalar.activation(ex[:], ee[:], act.Exp, accum_out=dn[:])
                nc.vector.tensor_scalar_max(dn[:], dn[:], 1e-30)
                rv = spool.tile([128, 1], dt.float32, tag="rv0")
                nc.vector.reciprocal(rv[:], dn[:])
                acc = wpool.tile([128, D0], dt.float16, tag="acc0")
                nc.vector.tensor_scalar(acc[:], gv[:, 0, 0:D0], ex[:, 0:1], None,
                                        op.mult)
                for k in range(1, K):
                    nc.vector.scalar_tensor_tensor(
                        out=acc[:], in0=gv[:, k, 0:D0], scalar=ex[:, k:k + 1],
                        in1=acc[:], op0=op.mult, op1=op.add)
                h1 = wpool.tile([128, D0], dt.float16, tag="h1")
                nc.vector.scalar_tensor_tensor(
                    out=h1[:], in0=acc[:], scalar=rv[:], in1=b0_sb[:],
                    op0=op.mult, op1=op.add)
                nc.scalar.activation(h1[:], h1[:], act.Relu)
                # hp1/el1/er1 = h1 @ W1ext via PE transpose
                pt1 = ppool.tile([128, 128], dt.float16, space="PSUM", tag="pt1")
                nc.tensor.transpose(pt1[:], h1[:, 0:128], id_sb[:])
                pt2 = ppool.tile([128, 128], dt.float16, space="PSUM", tag="pt2")
                nc.tensor.transpose(pt2[0:12, :], h1[:, 128:140], id_sb[:])
                t1s = wpool.tile([128, 128], dt.float16, tag="t1s")
                nc.vector.tensor_copy(t1s[:], pt1[:])
                t2s = wpool.tile([128, 128], dt.float16, tag="t2s")
                nc.vector.tensor_copy(t2s[0:12, :], pt2[0:12, :])
                php = ppool.tile([128, D1 + 2], dt.float32, space="PSUM", tag="php")
                nc.tensor.matmul(php[:], t1s[:], w1a_sb[:], start=True, stop=False)
                nc.tensor.matmul(php[:], t2s[0:12, :], w1b_sb[0:12, :],
                                 start=False, stop=True)
                hx1 = hexpool.tile([128, W1C], dt.float16, tag="hex1")
                nc.vector.tensor_copy(hx1[:, 0:D1 + 2], php[:])
                nc.vector.memset(hx1[:, D1 + 2:W1C], 0.0)
                if g == G - 1:
                    nc.vector.tensor_tensor(hx1[:, 7:8], hx1[:, 7:8],
                                            sm_sb[:], op=op.add)
                nc.vector.tensor_copy(er1_all[:, g:g + 1], php[:, 8:9])
                nc.sync.dma_start(shard1[:].rearrange("(g p) w -> g p w", p=128)[g],
                                  hx1[:])

            nc.gpsimd.collective_compute(
                "AllGather", op.bypass, replica_groups=[list(range(NCORES))],
                ins=[shard1[:]], outs=[table1[:]])

            # ---- Phase E1 ----
            for g in range(G):
                K = int(Kg[g])
                gt = gpool.tile([128, K * W1C], dt.float16, tag="g1")
                gv = gt[:].rearrange("p (k w) -> p k w", w=W1C)
                nc.sync.dma_start(
                    gv[:, 0], shard1[:].rearrange("(g p) w -> g p w", p=128)[g])
                for k in range(1, K):
                    nc.gpsimd.indirect_dma_start(
                        out=gv[:, k], out_offset=None, in_=table1[:],
                        in_offset=bass.IndirectOffsetOnAxis(
                            ap=of1_sb[:, cums[g] + k:cums[g] + k + 1], axis=0))
                ep = spool.tile([128, K], dt.float32, tag="ep1")
                nc.vector.tensor_scalar(ep[:], gv[:, :, 7], er1_all[:, g:g + 1],
                                        None, op.add)
                ee = spool.tile([128, K], dt.float32, tag="ee1")
                nc.vector.scalar_tensor_tensor(
                    out=ee[:], in0=ep[:], scalar=0.2, in1=ep[:],
                    op0=op.mult, op1=op.max)
                ex = spool.tile([128, K], dt.float32, tag="ex1")
                dn = spool.tile([128, 1], dt.float32, tag="dn1")
                nc.scalar.activation(ex[:], ee[:], act.Exp, accum_out=dn[:])
                nc.vector.tensor_scalar_max(dn[:], dn[:], 1e-30)
                rv = spool.tile([128, 1], dt.float32, tag="rv1")
                nc.vector.reciprocal(rv[:], dn[:])
                acc = spool.tile([128, D1], dt.float16, tag="acc1")
                nc.vector.tensor_scalar(acc[:], gv[:, 0, 0:D1], ex[:, 0:1], None,
                                        op.mult)
                for k in range(1, K):
                    nc.vector.scalar_tensor_tensor(
                        out=acc[:], in0=gv[:, k, 0:D1], scalar=ex[:, k:k + 1],
                        in1=acc[:], op0=op.mult, op1=op.add)
                ot = spool.tile([128, D1], dt.float32, tag="ot")
                nc.vector.scalar_tensor_tensor(
                    out=ot[:], in0=acc[:], scalar=rv[:], in1=b1_sb[:],
                    op0=op.mult, op1=op.add)
                nc.scalar.activation(ot[:], ot[:], act.Relu)
                nc.sync.dma_start(t_out[:].rearrange("(g p) w -> g p w", p=128)[g],
                                  ot[:])
    nc.compile()
    return nc


def kernel(**inputs):
    percore, Kg, cums = _host_prep(inputs["src"], inputs["dst"])
    com, xts = _prep_weights(inputs, percore)

    key = tuple(Kg)
    if key not in _CACHE:
        _CACHE[key] = _build(Kg, cums)
    nc = _CACHE[key]

    in_maps = []
    for c in range(NCORES):
        pc = percore[c]
        m = dict(x_up=xts[c], w0r=com["W0r"], w1a=com["W1a"], w1b=com["W1b"],
                 b0b=com["b0b"], b1b=com["b1b"],
                 ident=com["ident"], offs0=pc["offs0"], offs1=pc["offs1"],
                 sent_mask=com["sent_mask"])
        in_maps.append(m)

    from concourse.bass_utils import run_bass_kernel_spmd
    global LAST_EXEC_NS
    res = run_bass_kernel_spmd(nc, in_maps, core_ids=list(range(NCORES)),
                               trace=TRACE)
    LAST_EXEC_NS = res.exec_time_ns
    out = np.zeros((N, D1), dtype=np.float32)
    for c in range(NCORES):
        ob = res.results[c]["out_buf"]
        out[c * SHARD + percore[c]["order"]] = ob[:SHARD]
    return out


# revision 32
# speedup vs baseline: 1.2773x; 1.0041x over previous
"""Trainium2 Bass kernel for 2-layer single-head GAT (nn_GAT__80942953660642).

Strategy (8 NeuronCores, SPMD):
  - Nodes sharded contiguously: core c owns nodes [c*12500, (c+1)*12500).
  - Phase M: h_ext = X_shard @ [W0 | W0@al0 | W0@ar0] on PE (fp16 inputs,
    fp32 PSUM) -> rows [h(140), el, er, pad] packed into a 144-fp16 shard
    table.
  - AllGather the fp16 shard tables -> full node table per core.
  - Phase E0 (edge phase): per core, its dst nodes are degree-sorted into 98
    groups of 128 (one dst per partition). Each dst's incoming edges occupy
    padded slot columns; ALL K slots of a group are fetched with ONE batched
    indirect DMA (offset AP [128, K] -> out [128, K*144]), amortizing the
    ~1us SWDGE fixed cost. Edge softmax without max-subtraction; padding
    slots point at sentinel rows with el = -60000 so exp() kills them.
    Weighted accumulation via fused DVE multiply-add (fp16 data, fp32
    per-partition scalars).
  - hp1/el1/er1 = h1 @ [W1 | W1@al1 | W1@ar1] via PE transpose + matmul,
    second 16-fp16 table, AllGather, Phase E1 repeats at width 7.
  - Host assembles the final [100000, 7] fp32 output (inverse degree-sort).
"""
import sys
sys.path.insert(0, "/opt/trn_rl_repo")
import numpy as np

N = 100000
NCORES = 8
SHARD = 12500
PSHARD = 12544          # 98 * 128
G = PSHARD // 128       # 98 groups
KDIM = 1536             # 1433 padded to 12*128
D0 = 140
D0E = 142               # h(140), el, er
D1 = 7
W0C = 144               # L0 table row: h(140), el(140), er(141), pad
W1C = 16                # L1 table row: hp1(7), el1(7), er1(8), pad
SENT = np.float16(-60000.0)

_CACHE = {}
TRACE = False          # test harness sets this to capture an NTFF profile
LAST_EXEC_NS = None


def _host_prep(src, dst):
    src = np.asarray(src).astype(np.int64)
    dst = np.asarray(dst).astype(np.int64)
    deg = np.bincount(dst, minlength=N)
    nodes = np.arange(N, dtype=np.int64)
    pad_id = (nodes // SHARD) * PSHARD + (nodes % SHARD)  # original-order padded id

    percore = []
    Kg = np.zeros(G, dtype=np.int64)
    for c in range(NCORES):
        lo = c * SHARD
        m = (dst >= lo) & (dst < lo + SHARD)
        e_dst = dst[m] - lo
        e_src = src[m]
        d = deg[lo:lo + SHARD]
        order = np.argsort(-d, kind="stable")
        rank = np.empty(SHARD, dtype=np.int64)
        rank[order] = np.arange(SHARD)
        dsort = np.concatenate([d[order], np.zeros(PSHARD - SHARD, np.int64)])
        for g in range(G):
            Kg[g] = max(Kg[g], max(1, dsort[g * 128:(g + 1) * 128].max()))
        percore.append(dict(order=order, rank=rank, e_dst=e_dst, e_src=e_src))

    pos1 = np.empty(N, dtype=np.int64)  # sorted-space padded id
    for c in range(NCORES):
        pos1[c * SHARD:(c + 1) * SHARD] = c * PSHARD + percore[c]["rank"]

    SK = int(Kg.sum())
    cums = np.concatenate([[0], np.cumsum(Kg)])
    for c in range(NCORES):
        pc = percore[c]
        sent_row = c * PSHARD + PSHARD - 1
        r = pc["rank"][pc["e_dst"]]
        # self-loop edges first within each dst -> they land in slot 0
        not_self = (pc["e_src"] != pc["e_dst"] + c * SHARD).astype(np.int64)
        ordr = np.lexsort((not_self, r))
        r_s = r[ordr]
        kpos = np.arange(len(r_s)) - np.searchsorted(r_s, r_s)
        Kcap = int(Kg.max())
        slots0 = np.full((PSHARD, Kcap), sent_row, dtype=np.int64)
        slots0[r_s, kpos] = pos1[pc["e_src"][ordr]]
        slots1 = np.full((PSHARD, Kcap), sent_row, dtype=np.int64)
        slots1[r_s, kpos] = pos1[pc["e_src"][ordr]]
        # pack per-group [128, Kg[g]] -> [128, SK] (row offsets)
        offs0 = np.zeros((128, SK), np.int32)
        offs1 = np.zeros((128, SK), np.int32)
        for g in range(G):
            offs0[:, cums[g]:cums[g + 1]] = slots0[g * 128:(g + 1) * 128, :Kg[g]]
            offs1[:, cums[g]:cums[g + 1]] = slots1[g * 128:(g + 1) * 128, :Kg[g]]
        pc["offs0"] = offs0
        pc["offs1"] = offs1
    return percore, Kg.astype(int), cums.astype(int)


def _prep_weights(inputs, percore):
    X = np.asarray(inputs["X"], np.float32)
    W0 = np.asarray(inputs["W0"], np.float64)
    al0 = np.asarray(inputs["al0"], np.float64)
    ar0 = np.asarray(inputs["ar0"], np.float64)
    b0 = np.asarray(inputs["b0"], np.float32)
    W1 = np.asarray(inputs["W1"], np.float64)
    al1 = np.asarray(inputs["al1"], np.float64)
    ar1 = np.asarray(inputs["ar1"], np.float64)
    b1 = np.asarray(inputs["b1"], np.float32)

    # W0ext = [W0 | W0@al0 | W0@ar0]: h, el, er from one matmul
    W0e = np.concatenate([W0, (W0 @ al0)[:, None], (W0 @ ar0)[:, None]], axis=1)
    W0p = np.zeros((KDIM, D0E), np.float16)
    W0p[:1433] = W0e.astype(np.float16)
    # W0r[kp, k*142+j] = W0p[k*128+kp, j]
    W0r = np.ascontiguousarray(
        W0p.reshape(12, 128, D0E).transpose(1, 0, 2).reshape(128, 12 * D0E))
    # W1ext = [W1 | W1@al1 | W1@ar1] -> [140, 9]
    W1e = np.concatenate([W1, (W1 @ al1)[:, None], (W1 @ ar1)[:, None]],
                         axis=1).astype(np.float16)
    W1a = np.zeros((128, D1 + 2), np.float16)
    W1a[:128] = W1e[:128]
    W1b = np.zeros((128, D1 + 2), np.float16)
    W1b[:12] = W1e[128:140]
    bc = lambda v, w, dt: np.broadcast_to(np.asarray(v, dt)[None, :],
                                          (128, w)).copy()
    ident = np.eye(128, dtype=np.float16)
    sent_mask = np.zeros((128, 1), np.float16)
    sent_mask[SHARD - (G - 1) * 128:, 0] = SENT   # partitions 84.. are pads
    com = dict(W0r=W0r, W1a=W1a, W1b=W1b, sent_mask=sent_mask,
               b0b=bc(b0, D0, np.float16), b1b=bc(b1, D1, np.float32),
               ident=ident)

    # X tiles per core (fp16): xt[n, kp, k*128+nf] = X[lo + n*128+nf, k*128+kp]
    xts = []
    for c in range(NCORES):
        lo = c * SHARD
        Xp = np.zeros((PSHARD, KDIM), np.float16)
        Xp[:SHARD, :1433] = X[lo + percore[c]['order'], :].astype(np.float16)
        xt = Xp.reshape(G, 128, 12, 128).transpose(0, 3, 2, 1).reshape(G, 128, 12 * 128)
        xt = np.concatenate([xt, np.zeros((100 - G, 128, 12 * 128), np.float16)])
        xt = np.ascontiguousarray(
            xt.reshape(25, 4, 128, 1536).transpose(0, 2, 1, 3).reshape(25, 128, 4 * 1536))
        xts.append(xt)
    return com, xts


def _build(Kg, cums):
    import concourse.bass as bass
    import concourse.tile as tile
    from concourse import bacc, mybir
    dt = mybir.dt
    op = mybir.AluOpType
    act = mybir.ActivationFunctionType

    SK = int(sum(Kg))
    nc = bacc.Bacc("TRN2", target_bir_lowering=False, debug=False,
                   num_devices=NCORES)
    t_x = nc.dram_tensor("x_up", [25, 128, 4 * 12 * 128], dt.float16, kind="ExternalInput")
    t_w0 = nc.dram_tensor("w0r", [128, 12 * D0E], dt.float16, kind="ExternalInput")
    t_w1a = nc.dram_tensor("w1a", [128, D1 + 2], dt.float16, kind="ExternalInput")
    t_w1b = nc.dram_tensor("w1b", [128, D1 + 2], dt.float16, kind="ExternalInput")
    t_b0 = nc.dram_tensor("b0b", [128, D0], dt.float16, kind="ExternalInput")
    t_b1 = nc.dram_tensor("b1b", [128, D1], dt.float32, kind="ExternalInput")
    t_id = nc.dram_tensor("ident", [128, 128], dt.float16, kind="ExternalInput")
    t_of0 = nc.dram_tensor("offs0", [128, SK], dt.int32, kind="ExternalInput")
    t_of1 = nc.dram_tensor("offs1", [128, SK], dt.int32, kind="ExternalInput")
    t_sm = nc.dram_tensor("sent_mask", [128, 1], dt.float16, kind="ExternalInput")
    t_out = nc.dram_tensor("out_buf", [PSHARD, D1], dt.float32, kind="ExternalOutput")

    with tile.TileContext(nc) as tc:
        with tc.tile_pool(name="const", bufs=1) as cpool, \
             tc.tile_pool(name="xload", bufs=2) as xpool, \
             tc.tile_pool(name="hex", bufs=4) as hexpool, \
             tc.tile_pool(name="gath", bufs=4) as gpool, \
             tc.tile_pool(name="work", bufs=3) as wpool, \
             tc.tile_pool(name="small", bufs=6) as spool, \
             tc.tile_pool(name="psum", bufs=2, space="PSUM") as ppool, \
             tc.tile_pool(name="dram", bufs=1, space="DRAM") as dpool:

            # constants
            w0_sb = cpool.tile([128, 12 * D0E], dt.float16)
            nc.sync.dma_start(w0_sb[:], t_w0[:])
            w1a_sb = cpool.tile([128, D1 + 2], dt.float16)
            nc.sync.dma_start(w1a_sb[:], t_w1a[:])
            w1b_sb = cpool.tile([128, D1 + 2], dt.float16)
            nc.sync.dma_start(w1b_sb[:], t_w1b[:])
            b0_sb = cpool.tile([128, D0], dt.float16)
            nc.sync.dma_start(b0_sb[:], t_b0[:])
            b1_sb = cpool.tile([128, D1], dt.float32)
            nc.sync.dma_start(b1_sb[:], t_b1[:])
            id_sb = cpool.tile([128, 128], dt.float16)
            nc.sync.dma_start(id_sb[:], t_id[:])
            of0_sb = cpool.tile([128, SK], dt.int32)
            nc.sync.dma_start(of0_sb[:], t_of0[:])
            of1_sb = cpool.tile([128, SK], dt.int32)
            nc.sync.dma_start(of1_sb[:], t_of1[:])
            sm_sb = cpool.tile([128, 1], dt.float16)
            nc.sync.dma_start(sm_sb[:], t_sm[:])
            er1_all = cpool.tile([128, G], dt.float32)

            shard0 = dpool.tile([PSHARD, W0C], dt.float16)
            table0 = dpool.tile([NCORES * PSHARD, W0C], dt.float16, addr_space="Shared")
            shard1 = dpool.tile([PSHARD, W1C], dt.float16)
            table1 = dpool.tile([NCORES * PSHARD, W1C], dt.float16, addr_space="Shared")

            # ---- Phase M: h_ext = X @ W0ext, pack [h, el, er] rows ----
            XB = 4                      # X tiles per DMA (amortize fixed cost)
            xts = {}
            for n in range(G):
                b, t = n // XB, n % XB
                if t == 0:
                    xtile = xpool.tile([128, 4 * 12 * 128], dt.float16, tag="xt")
                    xts[b] = xtile
                    nc.sync.dma_start(xtile[:], t_x[:][b])
                xt = xts[b][:, t * 1536:(t + 1) * 1536]
                ph = ppool.tile([128, D0E], dt.float32, space="PSUM")
                for k in range(12):
                    nc.tensor.matmul(ph[:], xt[:, k * 128:(k + 1) * 128],
                                     w0_sb[:, k * D0E:(k + 1) * D0E],
                                     start=(k == 0), stop=(k == 11))
                hx = hexpool.tile([128, W0C], dt.float16, tag="hex0")
                nc.vector.tensor_copy(hx[:, 0:D0E], ph[:])
                nc.vector.memset(hx[:, D0E:W0C], 0.0)
                if n == G - 1:
                    nc.vector.tensor_tensor(hx[:, 140:141], hx[:, 140:141],
                                            sm_sb[:], op=op.add)
                nc.sync.dma_start(shard0[:].rearrange("(g p) w -> g p w", p=128)[n],
                                  hx[:])

            nc.gpsimd.collective_compute(
                "AllGather", op.bypass, replica_groups=[list(range(NCORES))],
                ins=[shard0[:]], outs=[table0[:]])

            # ---- Phase E0 ----
            for g in range(G):
                K = int(Kg[g])
                gt = gpool.tile([128, K * W0C], dt.float16, tag="g0")
                gv = gt[:].rearrange("p (k w) -> p k w", w=W0C)
                # slot 0 is the self-loop: contiguous rows of our own shard
                nc.sync.dma_start(
                    gv[:, 0], shard0[:].rearrange("(g p) w -> g p w", p=128)[g])
                for k in range(1, K):
                    nc.gpsimd.indirect_dma_start(
                        out=gv[:, k], out_offset=None, in_=table0[:],
                        in_offset=bass.IndirectOffsetOnAxis(
                            ap=of0_sb[:, cums[g] + k:cums[g] + k + 1], axis=0))
                # slot 0 is the self-loop -> its row IS the dst row; er = col 141
                er0 = spool.tile([128, 1], dt.float32, tag="er0")
                nc.vector.tensor_copy(er0[:], gv[:, 0, 141:142])
                ep = spool.tile([128, K], dt.float32, tag="ep0")
                nc.vector.tensor_scalar(ep[:], gv[:, :, 140], er0[:],
                                        None, op.add)
                ee = spool.tile([128, K], dt.float32, tag="ee0")
                nc.vector.scalar_tensor_tensor(
                    out=ee[:], in0=ep[:], scalar=0.2, in1=ep[:],
                    op0=op.mult, op1=op.max)
                ex = spool.tile([128, K], dt.float32, tag="ex0")
                dn = spool.tile([128, 1], dt.float32, tag="dn0")
                nc.scalar.activation(ex[:], ee[:], act.Exp, accum_out=dn[:])
                nc.vector.tensor_scalar_max(dn[:], dn[:], 1e-30)
                rv = spool.tile([128, 1], dt.float32, tag="rv0")
                nc.vector.reciprocal(rv[:], dn[:])
                acc = wpool.tile([128, D0], dt.float16, tag="acc0")
                nc.vector.tensor_scalar(acc[:], gv[:, 0, 0:D0], ex[:, 0:1], None,
                                        op.mult)
                for k in range(1, K):
                    nc.vector.scalar_tensor_tensor(
                        out=acc[:], in0=gv[:, k, 0:D0], scalar=ex[:, k:k + 1],
                        in1=acc[:], op0=op.mult, op1=op.add)
                h1 = wpool.tile([128, D0], dt.float16, tag="h1")
                nc.vector.scalar_tensor_tensor(
                    out=h1[:], in0=acc[:], scalar=rv[:], in1=b0_sb[:],
                    op0=op.mult, op1=op.add)
                nc.scalar.activation(h1[:], h1[:], act.Relu)
                # hp1/el1/er1 = h1 @ W1ext via PE transpose
                pt1 = ppool.tile([128, 128], dt.float16, space="PSUM", tag="pt1")
                nc.tensor.transpose(pt1[:], h1[:, 0:128], id_sb[:])
                pt2 = ppool.tile([128, 128], dt.float16, space="PSUM", tag="pt2")
                nc.tensor.transpose(pt2[0:12, :], h1[:, 128:140], id_sb[:])
                t1s = wpool.tile([128, 128], dt.float16, tag="t1s")
                nc.vector.tensor_copy(t1s[:], pt1[:])
                t2s = wpool.tile([128, 128], dt.float16, tag="t2s")
                nc.vector.tensor_copy(t2s[0:12, :], pt2[0:12, :])
                php = ppool.tile([128, D1 + 2], dt.float32, space="PSUM", tag="php")
                nc.tensor.matmul(php[:], t1s[:], w1a_sb[:], start=True, stop=False)
                nc.tensor.matmul(php[:], t2s[0:12, :], w1b_sb[0:12, :],
                                 start=False, stop=True)
                hx1 = hexpool.tile([128, W1C], dt.float16, tag="hex1")
                nc.vector.tensor_copy(hx1[:, 0:D1 + 2], php[:])
                nc.vector.memset(hx1[:, D1 + 2:W1C], 0.0)
                if g == G - 1:
                    nc.vector.tensor_tensor(hx1[:, 7:8], hx1[:, 7:8],
                                            sm_sb[:], op=op.add)
                nc.vector.tensor_copy(er1_all[:, g:g + 1], php[:, 8:9])
                nc.sync.dma_start(shard1[:].rearrange("(g p) w -> g p w", p=128)[g],
                                  hx1[:])

            nc.gpsimd.collective_compute(
                "AllGather", op.bypass, replica_groups=[list(range(NCORES))],
                ins=[shard1[:]], outs=[table1[:]])

            # ---- Phase E1 ----
            for g in range(G):
                K = int(Kg[g])
                gt = gpool.tile([128, K * W1C], dt.float16, tag="g1")
                gv = gt[:].rearrange("p (k w) -> p k w", w=W1C)
                nc.sync.dma_start(
                    gv[:, 0], shard1[:].rearrange("(g p) w -> g p w", p=128)[g])
                for k in range(1, K):
                    nc.gpsimd.indirect_dma_start(
                        out=gv[:, k], out_offset=None, in_=table1[:],
                        in_offset=bass.IndirectOffsetOnAxis(
                            ap=of1_sb[:, cums[g] + k:cums[g] + k + 1], axis=0))
                ep = spool.tile([128, K], dt.float32, tag="ep1")
                nc.vector.tensor_scalar(ep[:], gv[:, :, 7], er1_all[:, g:g + 1],
                                        None, op.add)
                ee = spool.tile([128, K], dt.float32, tag="ee1")
                nc.vector.scalar_tensor_tensor(
                    out=ee[:], in0=ep[:], scalar=0.2, in1=ep[:],
                    op0=op.mult, op1=op.max)
                ex = spool.tile([128, K], dt.float32, tag="ex1")
                dn = spool.tile([128, 1], dt.float32, tag="dn1")
                nc.scalar.activation(ex[:], ee[:], act.Exp, accum_out=dn[:])
                nc.vector.tensor_scalar_max(dn[:], dn[:], 1e-30)
                rv = spool.tile([128, 1], dt.float32, tag="rv1")
                nc.vector.reciprocal(rv[:], dn[:])
                acc = spool.tile([128, D1], dt.float16, tag="acc1")
                nc.vector.tensor_scalar(acc[:], gv[:, 0, 0:D1], ex[:, 0:1], None,
                                        op.mult)
                for k in range(1, K):
                    nc.vector.scalar_tensor_tensor(
                        out=acc[:], in0=gv[:, k, 0:D1], scalar=ex[:, k:k + 1],
                        in1=acc[:], op0=op.mult, op1=op.add)
                ot = spool.tile([128, D1], dt.float32, tag="ot")
                nc.vector.scalar_tensor_tensor(
                    out=ot[:], in0=acc[:], scalar=rv[:], in1=b1_sb[:],
                    op0=op.mult, op1=op.add)
                nc.scalar.activation(ot[:], ot[:], act.Relu)
                nc.sync.dma_start(t_out[:].rearrange("(g p) w -> g p w", p=128)[g],
                                  ot[:])
    nc.compile()
    return nc


def kernel(**inputs):
    percore, Kg, cums = _host_prep(inputs["src"], inputs["dst"])
    com, xts = _prep_weights(inputs, percore)

    key = tuple(Kg)
    if key not in _CACHE:
        _CACHE[key] = _build(Kg, cums)
    nc = _CACHE[key]

    in_maps = []
    for c in range(NCORES):
        pc = percore[c]
        m = dict(x_up=xts[c], w0r=com["W0r"], w1a=com["W1a"], w1b=com["W1b"],
                 b0b=com["b0b"], b1b=com["b1b"],
                 ident=com["ident"], offs0=pc["offs0"], offs1=pc["offs1"],
                 sent_mask=com["sent_mask"])
        in_maps.append(m)

    from concourse.bass_utils import run_bass_kernel_spmd
    global LAST_EXEC_NS
    res = run_bass_kernel_spmd(nc, in_maps, core_ids=list(range(NCORES)),
                               trace=TRACE)
    LAST_EXEC_NS = res.exec_time_ns
    out = np.zeros((N, D1), dtype=np.float32)
    for c in range(NCORES):
        ob = res.results[c]["out_buf"]
        out[c * SHARD + percore[c]["order"]] = ob[:SHARD]
    return out
